# revision 11
# baseline (speedup 1.0000x reference)
"""DeformConv2d (B=8, C=128, H=W=64, K=3x3, pad 1, stride 1) on 8 trn2 NeuronCores.

Data-parallel over batch: core b handles image b. Per core:
  - The image is staged host-side in DRAM as XR[r] = [pix r | pix r+1] bf16
    rows (channel-minor, 512B each) over the 68x68 zero-ring-padded grid, so
    the reference's out-of-bounds corner masking is exactly reproduced by
    clamped sample indices landing in the zero ring.
  - Bilinear corners are fetched with SWDGE dma_gather (transpose mode): each
    index pulls one 512B row-pair from DRAM and lands it channel-major in
    SBUF. Two gathers per (tap, block) (y0 / y0+1 row-pairs, same index
    tensor with a +68-row base offset) produce the 4 bilinear corner planes
    [A|B|C|D] in [cin, pos] layout. Blocks are 896 positions (the SWDGE
    descriptor ring caps one transpose gather at ~1008 indices).
  - Bilinear weights (pure fraction products; no masks needed) are computed
    on DVE in a [128, K*Q] gen layout (position%128 on partitions), staged to
    DRAM block-major, and broadcast to all 128 partitions once per tap (4MB
    stride-0-source DMA, alternating issue queues).
  - Main loop is tap-major: per (tap, block) one DVE multiply forms the 4
    weighted corner planes; PE matmuls accumulate every (tap, plane) into a
    single whole-image PSUM region (1 fp32 per position, 8 banks).
  - Tail: bias add per quarter -> fp32 output.
"""
import numpy as np
import ml_dtypes

B, CIN, H, W = 8, 128, 64, 64
COUT, KH, KW = 128, 3, 3
K = KH * KW
HO, WO = 64, 64
P = 128                      # partitions
NPOS = HO * WO               # 4096 output positions per image
Q = NPOS // P                # 32 free-dim columns in the [128, 288] gen layout
PADR = 2                     # zero-pad ring width
HP = H + 2 * PADR            # 68
WP = W + 2 * PADR            # 68
NROW = HP * WP               # 4624 padded pixel rows
NRALLOC = NROW + 2           # xr rows incl. tail pad for the 2-row elem window
FB = 1024.0                  # floor-trick bias constant
# gather blocks: <=896 idxs per dma_gather (SWDGE ring cap)
BLKQ = [7, 7, 7, 7, 4]       # q-groups (128 pos) per block: 896*4 + 512
BLK0 = [0, 7, 14, 21, 28]    # q-group offsets


def _bank_pieces(c0, n):
    """Split [c0, c0+n) at 512-col PSUM bank boundaries."""
    out = []
    c = c0
    while c < c0 + n:
        hi = min((c // 512 + 1) * 512, c0 + n)
        out.append((c, hi - c))
        c = hi
    return out


def _build_kernel():
    import concourse.bacc as bacc
    import concourse.mybir as mybir
    import concourse.tile as tile
    import concourse.library_config as library_config

    nc = bacc.Bacc("TRN2", target_bir_lowering=False, debug=False, num_devices=8)
    f32, bf16, i16 = mybir.dt.float32, mybir.dt.bfloat16, mybir.dt.int16
    ALU = mybir.AluOpType

    xr_d = nc.dram_tensor("xr", [NRALLOC, 2 * P], bf16, kind="ExternalInput")
    off_d = nc.dram_tensor("offs", [P, 2 * K * Q], f32, kind="ExternalInput")
    wmat_d = nc.dram_tensor("wmat", [P, K * COUT], bf16, kind="ExternalInput")
    bias_d = nc.dram_tensor("bias", [P, 1], f32, kind="ExternalInput")
    hob_d = nc.dram_tensor("hob", [P, K * Q], f32, kind="ExternalInput")
    wob_d = nc.dram_tensor("wob", [P, K * Q], f32, kind="ExternalInput")
    eye_d = nc.dram_tensor("eye", [P, P], bf16, kind="ExternalInput")
    out_d = nc.dram_tensor("out", [P, NPOS], f32, kind="ExternalOutput")

    with tile.TileContext(nc) as tc:
        with tc.tile_pool(name="const", bufs=1) as cpool, \
             tc.tile_pool(name="gen", bufs=1) as gpool, \
             tc.tile_pool(name="wbc", bufs=2) as wpool, \
             tc.tile_pool(name="gath", bufs=3) as gapool, \
             tc.tile_pool(name="mm", bufs=3) as mpool, \
             tc.tile_pool(name="outp", bufs=2) as opool, \
             tc.tile_pool(name="dramw", bufs=1, space="DRAM") as dpool, \
             tc.tile_pool(name="ps", bufs=1, space="PSUM") as pspool:

            # staging for weight planes: block-major, within a block
            # (j, qw, p) so a (tap, block) slice is contiguous
            wrow = dpool.tile([K, 4 * NPOS], mybir.dt.bfloat16)

            nc.gpsimd.load_library(library_config.mlp)

            # -------------- stage 0: loads ------------------------------
            wmat = cpool.tile([P, K * COUT], bf16)
            nc.sync.dma_start(out=wmat[:], in_=wmat_d.ap())
            bias = cpool.tile([P, 1], f32)
            nc.sync.dma_start(out=bias[:], in_=bias_d.ap())
            hob = cpool.tile([P, K * Q], f32)
            nc.sync.dma_start(out=hob[:], in_=hob_d.ap())
            wob = cpool.tile([P, K * Q], f32)
            nc.sync.dma_start(out=wob[:], in_=wob_d.ap())
            eye = cpool.tile([P, P], bf16)
            nc.sync.dma_start(out=eye[:], in_=eye_d.ap())
            offyx = cpool.tile([P, 2 * K * Q], f32)
            nc.sync.dma_start(out=offyx[:], in_=off_d.ap())
            offy = offyx[:, 0 : K * Q]
            offx = offyx[:, K * Q : 2 * K * Q]

            # ---------------- stage 1: weights + indices -----------------
            NG = K * Q  # 288
            pyb = gpool.tile([P, NG], f32)
            pxb = gpool.tile([P, NG], f32)
            # pyb = (offy + FB) + hob   (hob already holds ho - 1 + ky)
            nc.vector.scalar_tensor_tensor(
                out=pyb[:], in0=offy, scalar=FB, in1=hob[:],
                op0=ALU.add, op1=ALU.add)
            nc.vector.scalar_tensor_tensor(
                out=pxb[:], in0=offx, scalar=FB, in1=wob[:],
                op0=ALU.add, op1=ALU.add)
            # floor robust to cast rounding mode (trunc in sim, RN on hw):
            # y0 = cast(pyb); lyr = pyb - y0; adj = (lyr < 0); floor = y0 - adj
            def floor_frac(pb, sfx):
                i0 = gpool.tile([P, NG], mybir.dt.int32, tag="ffi" + sfx)
                nc.vector.tensor_copy(out=i0[:], in_=pb[:])
                f0 = gpool.tile([P, NG], f32, tag="fff" + sfx)
                nc.vector.tensor_copy(out=f0[:], in_=i0[:])
                lr = gpool.tile([P, NG], f32, tag="ffl" + sfx)
                nc.vector.tensor_tensor(out=lr[:], in0=pb[:], in1=f0[:],
                                        op=ALU.subtract)
                adj = gpool.tile([P, NG], f32, tag="ffa" + sfx)
                nc.vector.tensor_scalar(out=adj[:], in0=lr[:], scalar1=0.0,
                                        scalar2=None, op0=ALU.is_lt)
                fr = gpool.tile([P, NG], f32, tag="ffr" + sfx)
                nc.vector.tensor_tensor(out=fr[:], in0=lr[:], in1=adj[:],
                                        op=ALU.add)
                fl = gpool.tile([P, NG], f32, tag="ffo" + sfx)
                nc.vector.tensor_tensor(out=fl[:], in0=f0[:], in1=adj[:],
                                        op=ALU.subtract)
                return fl, fr
            y0f, ly = floor_frac(pyb, "y")
            x0f, lx = floor_frac(pxb, "x")
            omly = gpool.tile([P, NG], f32)
            omlx = gpool.tile([P, NG], f32)
            nc.vector.tensor_scalar(out=omly[:], in0=ly[:], scalar1=-1.0, scalar2=1.0,
                                    op0=ALU.mult, op1=ALU.add)
            nc.vector.tensor_scalar(out=omlx[:], in0=lx[:], scalar1=-1.0, scalar2=1.0,
                                    op0=ALU.mult, op1=ALU.add)
            # clamp biased corner coords to [-PADR, 64]+FB
            ycl = gpool.tile([P, NG], f32)
            xcl = gpool.tile([P, NG], f32)
            nc.vector.tensor_scalar(out=ycl[:], in0=y0f[:], scalar1=FB - PADR,
                                    scalar2=FB + 64.0, op0=ALU.max, op1=ALU.min)
            nc.vector.tensor_scalar(out=xcl[:], in0=x0f[:], scalar1=FB - PADR,
                                    scalar2=FB + 64.0, op0=ALU.max, op1=ALU.min)
            # row idx = (ycl-FB+PADR)*WP + (xcl-FB+PADR)
            linf = gpool.tile([P, NG], f32)
            nc.vector.scalar_tensor_tensor(
                out=linf[:], in0=ycl[:], scalar=float(WP), in1=xcl[:],
                op0=ALU.mult, op1=ALU.add)
            linf2 = gpool.tile([P, NG], f32)
            nc.vector.tensor_scalar(out=linf2[:], in0=linf[:],
                                    scalar1=-(WP + 1.0) * (FB - PADR),
                                    scalar2=None, op0=ALU.add)
            lin16 = gpool.tile([P, NG], i16)
            nc.vector.tensor_copy(out=lin16[:], in_=linf2[:])

            # weight plane products in (k, j, q) free order; per tap the
            # [128, 128] block (pos%128 x (j,q)) is PE-transposed so the DRAM
            # staging DMA is contiguous (256B/partition descriptors)
            wpre = gpool.tile([P, 4 * NG], bf16)
            wv = wpre[:].rearrange("p (k j q) -> p k j q", k=K, j=4, q=Q)
            omly3 = omly[:].rearrange("p (k q) -> p k q", k=K, q=Q)
            ly3 = ly[:].rearrange("p (k q) -> p k q", k=K, q=Q)
            omlx3 = omlx[:].rearrange("p (k q) -> p k q", k=K, q=Q)
            lx3 = lx[:].rearrange("p (k q) -> p k q", k=K, q=Q)
            nc.vector.tensor_tensor(out=wv[:, :, 0, :], in0=omly3, in1=omlx3,
                                    op=ALU.mult)  # wA = (1-ly)(1-lx)
            nc.vector.tensor_tensor(out=wv[:, :, 1, :], in0=omly3, in1=lx3,
                                    op=ALU.mult)  # wB = (1-ly)lx
            nc.vector.tensor_tensor(out=wv[:, :, 2, :], in0=ly3, in1=omlx3,
                                    op=ALU.mult)  # wC = ly(1-lx)
            nc.vector.tensor_tensor(out=wv[:, :, 3, :], in0=ly3, in1=lx3,
                                    op=ALU.mult)  # wD = ly lx
            # transpose each tap's [pos%128, (j,q)] block on PE (psum is free
            # in the gen phase), round-trip through SBUF, stage contiguously:
            # wrow[k, (j*Q + q)*128 + p]
            for k in range(K):
                pst = pspool.tile([P, NPOS], f32, tag="ps")
                nc.tensor.matmul(
                    pst[:, 0:P], wpre[:, k * P : (k + 1) * P], eye[:],
                    start=True, stop=True, skip_group_check=True)
                wtr = gpool.tile([P, P], bf16, tag="wtr")
                nc.vector.tensor_copy(out=wtr[:], in_=pst[:, 0:P])
                nc.sync.dma_start(
                    out=wrow[k : k + 1, :].rearrange("k (c p) -> k c p", c=P)[0],
                    in_=wtr[:])

            # index tensors: wrapped-16 layout for dma_gather, one tile per
            # tap so tap k's gathers only wait on tap k's build
            # idxk[k][16g + r, 8q + u] = lin16[16u + r, k*Q + q]
            idxk = []
            for k in range(K):
                t = gpool.tile([P, 8 * Q], i16, tag=f"idx{k}")
                e1 = nc.scalar if k % 2 == 0 else nc.sync
                e2 = nc.sync if k % 2 == 0 else nc.scalar
                for u in range(8):
                    e1.dma_start(
                        out=t[0:16, :].rearrange(
                            "p (q u) -> p q u", q=Q, u=8)[:, :, u],
                        in_=lin16[16 * u : 16 * u + 16,
                                  k * Q : (k + 1) * Q])
                for lo, n in [(16, 16), (32, 32), (64, 64)]:
                    e2.dma_start(out=t[lo : lo + n, :], in_=t[0:lo][0:n, :])
                idxk.append(t)

            # ------------- stage 2+3: tap-major gather/mul/matmul ----------
            # all 4 corner planes and 9 taps accumulate into the same psum
            # columns. PSUM `start` resets a whole 512-col bank, so each
            # block gets a bank-exclusive 1024-aligned stripe: blocks 0-3 in
            # generation A (8 banks), block 4 as a second 512-col generation.
            psum = pspool.tile([P, NPOS], f32, tag="ps")
            for k in range(K):
                # broadcast tap k's weight planes for blocks 0-3 (stride-0)
                wbck = wpool.tile([P, 4 * NPOS], bf16, tag="wb")
                eng = nc.sync if k % 2 == 0 else nc.scalar
                eng.dma_start(out=wbck[:],
                              in_=wrow[k : k + 1, :].to_broadcast((P, 4 * NPOS)))
                lhsT = wmat[:, k * COUT : (k + 1) * COUT]
                for blk in range(5):
                    bq, b0 = BLKQ[blk], BLK0[blk]
                    BS = bq * P  # positions in this block
                    g4 = gapool.tile([P, 4 * 896], bf16, tag="g")
                    nc.gpsimd.dma_gather(
                        g4[:, 0 : 2 * BS].rearrange("p (j n) -> p j n", j=2),
                        xr_d.ap(), idxk[k][:, 8 * b0 : 8 * (b0 + bq)],
                        BS, BS, 2 * P, transpose=True)
                    nc.gpsimd.dma_gather(
                        g4[:, 2 * BS : 4 * BS].rearrange("p (j n) -> p j n", j=2),
                        xr_d.ap()[WP:NRALLOC], idxk[k][:, 8 * b0 : 8 * (b0 + bq)],
                        BS, BS, 2 * P, transpose=True)
                    m = mpool.tile([P, 4 * 896], bf16, tag="m")
                    nc.vector.tensor_tensor(
                        out=m[:, 0 : 4 * BS].rearrange(
                            "p (j n) -> p j n", j=4),
                        in0=g4[:, 0 : 4 * BS].rearrange(
                            "p (j n) -> p j n", j=4),
                        in1=wbck[:].rearrange("p (j c) -> p j c", j=4)[
                            :, :, P * b0 : P * (b0 + bq)],
                        op=ALU.mult)
                    if blk < 4:
                        # bank-exclusive 1024-aligned stripe, chain owns banks
                        for j in range(4):
                            for s0, n in [(0, 512), (512, BS - 512)]:
                                nc.tensor.matmul(
                                    psum[:, 1024 * blk + s0 :
                                         1024 * blk + s0 + n],
                                    lhsT, m[:, j * BS + s0 : j * BS + s0 + n],
                                    start=(k == 0 and j == 0),
                                    stop=(k == K - 1 and j == 3),
                                    skip_group_check=True)
                    else:
                        # block 4 rides the zeroed stripe pads (cols 896-1023
                        # of banks 1,3,5,7): gen-A's start already zeroed the
                        # whole bank, so accumulate with start=False
                        for j in range(4):
                            for c in range(4):
                                nc.tensor.matmul(
                                    psum[:, 1024 * c + 896 : 1024 * c + 1024],
                                    lhsT,
                                    m[:, j * BS + 128 * c : j * BS + 128 * (c + 1)],
                                    start=False,
                                    stop=(k == K - 1 and j == 3),
                                    skip_group_check=True)
            # tails: bias add -> fp32 out (blocks 0-3 + the 4 pad chunks)
            for blk in range(4):
                bq, b0 = BLKQ[blk], BLK0[blk]
                BS = bq * P
                o = opool.tile([P, 896], f32, tag="o")
                nc.vector.tensor_scalar(
                    out=o[:, 0:BS], in0=psum[:, 1024 * blk : 1024 * blk + BS],
                    scalar1=bias[:, 0:1], scalar2=None, op0=ALU.add)
                nc.sync.dma_start(
                    out=out_d.ap()[:, P * b0 : P * (b0 + bq)], in_=o[:, 0:BS])
            for c in range(4):
                o = opool.tile([P, 896], f32, tag="o")
                nc.vector.tensor_scalar(
                    out=o[:, 0:128],
                    in0=psum[:, 1024 * c + 896 : 1024 * c + 1024],
                    scalar1=bias[:, 0:1], scalar2=None, op0=ALU.add)
                nc.scalar.dma_start(
                    out=out_d.ap()[:, 3584 + 128 * c : 3584 + 128 * (c + 1)],
                    in_=o[:, 0:128])

    nc.compile()
    return nc


_NC_CACHE = None


def _host_inputs(x, offset, weight, bias):
    """Per-core input maps (core b <- batch b) + replicated constants."""
    wq = np.ascontiguousarray(weight, np.float32)  # [COUT, CIN, KH, KW]
    # wmat[c, k*COUT + o] = weight[o, c, ky, kx]
    wmat = wq.reshape(COUT, CIN, K).transpose(1, 2, 0).reshape(CIN, K * COUT)
    wmat = np.ascontiguousarray(wmat).astype(ml_dtypes.bfloat16)
    bias_h = np.ascontiguousarray(bias, np.float32).reshape(P, 1)
    # hob[Pp, k*Q+q] = ho(p) - 1 + ky,  wob = wo(p) - 1 + kx,  p = q*128 + Pp
    p_of = (np.arange(Q)[:, None] * P + np.arange(P)[None, :])  # [Q, P]
    ho = (p_of // WO).astype(np.float32)
    wo = (p_of % WO).astype(np.float32)
    hob = np.empty((P, K * Q), np.float32)
    wob = np.empty((P, K * Q), np.float32)
    for k in range(K):
        hob[:, k * Q : (k + 1) * Q] = (ho + (k // 3 - 1)).T
        wob[:, k * Q : (k + 1) * Q] = (wo + (k % 3 - 1)).T
    in_maps = []
    for b in range(B):
        # padded channel-minor image rows with 2-px zero ring; each stored
        # row r holds pixels (r, r+1) so a gather elem is one 512B block
        xrp = np.zeros((NRALLOC + 1, P), ml_dtypes.bfloat16)
        img = np.ascontiguousarray(x[b], np.float32)  # [C, H, W]
        xrp[:NROW].reshape(HP, WP, P)[PADR:PADR + H, PADR:PADR + W, :] = (
            img.transpose(1, 2, 0).astype(ml_dtypes.bfloat16))
        xr = np.concatenate([xrp[:NRALLOC], xrp[1:NRALLOC + 1]], axis=1)
        xr = np.ascontiguousarray(xr)
        # permuted offsets: offyx[Pp, k*Q+q (+KQ for x)] = off[2k(+1), q*128+Pp]
        off = np.ascontiguousarray(offset[b], np.float32).reshape(2 * K, Q, P)
        offyx = np.empty((P, 2 * K * Q), np.float32)
        offyx[:, 0 : K * Q] = off[0::2].transpose(2, 0, 1).reshape(P, K * Q)
        offyx[:, K * Q : 2 * K * Q] = off[1::2].transpose(2, 0, 1).reshape(P, K * Q)
        in_maps.append({
            "xr": xr,
            "offs": offyx,
            "eye": np.eye(P, dtype=ml_dtypes.bfloat16),
            "wmat": wmat,
            "bias": bias_h,
            "hob": hob,
            "wob": wob,
        })
    return in_maps


def kernel(x, offset, weight, bias):
    global _NC_CACHE
    from concourse.bass_utils import run_bass_kernel_spmd

    if _NC_CACHE is None:
        _NC_CACHE = _build_kernel()
    nc = _NC_CACHE
    in_maps = _host_inputs(x, offset, weight, bias)
    res = None
    for attempt in range(3):
        try:
            res = run_bass_kernel_spmd(nc, in_maps, list(range(B)))
            break
        except Exception:
            if attempt == 2:
                raise
    out = np.stack([res.results[b]["out"].reshape(COUT, HO, WO) for b in range(B)])
    return out.astype(np.float32)


if __name__ == "__main__":
    import sys
    d = np.load("/tmp/inputs.npz")
    if len(sys.argv) > 1 and sys.argv[1] == "sim":
        from concourse.bass_interp import CoreSim
        nc = _build_kernel()
        in_maps = _host_inputs(d["x"], d["offset"], d["weight"], d["bias"])
        sim = CoreSim(nc)
        for kk, vv in in_maps[0].items():
            sim.tensor(kk)[:] = vv
        sim.simulate()
        out = np.asarray(sim.tensor("out")).reshape(1, COUT, HO, WO)
        exp = np.load("/tmp/expected.npy")[0:1]
    else:
        out = kernel(d["x"], d["offset"], d["weight"], d["bias"])
        exp = np.load("/tmp/expected.npy")
    err = np.abs(out - exp)
    print("rel l2:", np.linalg.norm(out - exp) / np.linalg.norm(exp))
    print("absmax rel:", err.max() / np.abs(exp).max())


# revision 12
# speedup vs baseline: 1.0218x; 1.0218x over previous
"""DeformConv2d (B=8, C=128, H=W=64, K=3x3, pad 1, stride 1) on 8 trn2 NeuronCores.

Data-parallel over batch: core b handles image b. Per core:
  - The image is staged host-side in DRAM as XR[r] = [pix r | pix r+1] bf16
    rows (channel-minor, 512B each) over the 68x68 zero-ring-padded grid, so
    the reference's out-of-bounds corner masking is exactly reproduced by
    clamped sample indices landing in the zero ring.
  - Bilinear corners are fetched with SWDGE dma_gather (transpose mode): each
    index pulls one 512B row-pair from DRAM and lands it channel-major in
    SBUF. Two gathers per (tap, block) (y0 / y0+1 row-pairs, same index
    tensor with a +68-row base offset) produce the 4 bilinear corner planes
    [A|B|C|D] in [cin, pos] layout. Blocks are 896 positions (the SWDGE
    descriptor ring caps one transpose gather at ~1008 indices).
  - Bilinear weights (pure fraction products; no masks needed) are computed
    on DVE in a [128, K*Q] gen layout (position%128 on partitions), staged to
    DRAM block-major, and broadcast to all 128 partitions once per tap (4MB
    stride-0-source DMA, alternating issue queues).
  - Main loop is tap-major: per (tap, block) one DVE multiply forms the 4
    weighted corner planes; PE matmuls accumulate every (tap, plane) into a
    single whole-image PSUM region (1 fp32 per position, 8 banks).
  - Tail: bias add per quarter -> fp32 output.
"""
import numpy as np
import ml_dtypes

B, CIN, H, W = 8, 128, 64, 64
COUT, KH, KW = 128, 3, 3
K = KH * KW
HO, WO = 64, 64
P = 128                      # partitions
NPOS = HO * WO               # 4096 output positions per image
Q = NPOS // P                # 32 free-dim columns in the [128, 288] gen layout
PADR = 2                     # zero-pad ring width
HP = H + 2 * PADR            # 68
WP = W + 2 * PADR            # 68
NROW = HP * WP               # 4624 padded pixel rows
NRALLOC = NROW + 2           # xr rows incl. tail pad for the 2-row elem window
FB = 1024.0                  # floor-trick bias constant
# gather blocks: <=896 idxs per dma_gather (SWDGE ring cap)
BLKQ = [7, 7, 7, 7, 4]       # q-groups (128 pos) per block: 896*4 + 512
BLK0 = [0, 7, 14, 21, 28]    # q-group offsets


def _bank_pieces(c0, n):
    """Split [c0, c0+n) at 512-col PSUM bank boundaries."""
    out = []
    c = c0
    while c < c0 + n:
        hi = min((c // 512 + 1) * 512, c0 + n)
        out.append((c, hi - c))
        c = hi
    return out


def _build_kernel():
    import concourse.bacc as bacc
    import concourse.mybir as mybir
    import concourse.tile as tile
    import concourse.library_config as library_config

    nc = bacc.Bacc("TRN2", target_bir_lowering=False, debug=False, num_devices=8)
    f32, bf16, i16 = mybir.dt.float32, mybir.dt.bfloat16, mybir.dt.int16
    ALU = mybir.AluOpType

    xr_d = nc.dram_tensor("xr", [NRALLOC, 2 * P], bf16, kind="ExternalInput")
    off_d = nc.dram_tensor("offs", [P, 2 * K * Q], f32, kind="ExternalInput")
    wmat_d = nc.dram_tensor("wmat", [P, K * COUT], bf16, kind="ExternalInput")
    bias_d = nc.dram_tensor("bias", [P, 1], f32, kind="ExternalInput")
    hob_d = nc.dram_tensor("hob", [P, K * Q], f32, kind="ExternalInput")
    wob_d = nc.dram_tensor("wob", [P, K * Q], f32, kind="ExternalInput")
    eye_d = nc.dram_tensor("eye", [P, P], bf16, kind="ExternalInput")
    out_d = nc.dram_tensor("out", [P, NPOS], f32, kind="ExternalOutput")

    with tile.TileContext(nc) as tc:
        with tc.tile_pool(name="const", bufs=1) as cpool, \
             tc.tile_pool(name="gen", bufs=1) as gpool, \
             tc.tile_pool(name="wbc", bufs=2) as wpool, \
             tc.tile_pool(name="gath", bufs=3) as gapool, \
             tc.tile_pool(name="mm", bufs=3) as mpool, \
             tc.tile_pool(name="outp", bufs=2) as opool, \
             tc.tile_pool(name="dramw", bufs=1, space="DRAM") as dpool, \
             tc.tile_pool(name="ps", bufs=1, space="PSUM") as pspool:

            # staging for weight planes: block-major, within a block
            # (j, qw, p) so a (tap, block) slice is contiguous
            wrow = dpool.tile([K, 4 * NPOS], mybir.dt.bfloat16)

            nc.gpsimd.load_library(library_config.mlp)

            # -------------- stage 0: loads ------------------------------
            wmat = cpool.tile([P, K * COUT], bf16)
            nc.sync.dma_start(out=wmat[:], in_=wmat_d.ap())
            bias = cpool.tile([P, 1], f32)
            nc.sync.dma_start(out=bias[:], in_=bias_d.ap())
            hob = cpool.tile([P, K * Q], f32)
            nc.sync.dma_start(out=hob[:], in_=hob_d.ap())
            wob = cpool.tile([P, K * Q], f32)
            nc.sync.dma_start(out=wob[:], in_=wob_d.ap())
            eye = cpool.tile([P, P], bf16)
            nc.sync.dma_start(out=eye[:], in_=eye_d.ap())
            offyx = cpool.tile([P, 2 * K * Q], f32)
            nc.sync.dma_start(out=offyx[:], in_=off_d.ap())
            offy = offyx[:, 0 : K * Q]
            offx = offyx[:, K * Q : 2 * K * Q]

            # ---------------- stage 1: weights + indices -----------------
            NG = K * Q  # 288
            pyb = gpool.tile([P, NG], f32)
            pxb = gpool.tile([P, NG], f32)
            # pyb = (offy + FB) + hob   (hob already holds ho - 1 + ky)
            nc.vector.scalar_tensor_tensor(
                out=pyb[:], in0=offy, scalar=FB, in1=hob[:],
                op0=ALU.add, op1=ALU.add)
            nc.vector.scalar_tensor_tensor(
                out=pxb[:], in0=offx, scalar=FB, in1=wob[:],
                op0=ALU.add, op1=ALU.add)
            # floor robust to cast rounding mode (trunc in sim, RN on hw):
            # y0 = cast(pyb); lyr = pyb - y0; adj = (lyr < 0); floor = y0 - adj
            def floor_frac(pb, sfx):
                i0 = gpool.tile([P, NG], mybir.dt.int32, tag="ffi" + sfx)
                nc.vector.tensor_copy(out=i0[:], in_=pb[:])
                f0 = gpool.tile([P, NG], f32, tag="fff" + sfx)
                nc.vector.tensor_copy(out=f0[:], in_=i0[:])
                lr = gpool.tile([P, NG], f32, tag="ffl" + sfx)
                nc.vector.tensor_tensor(out=lr[:], in0=pb[:], in1=f0[:],
                                        op=ALU.subtract)
                adj = gpool.tile([P, NG], f32, tag="ffa" + sfx)
                nc.vector.tensor_scalar(out=adj[:], in0=lr[:], scalar1=0.0,
                                        scalar2=None, op0=ALU.is_lt)
                fr = gpool.tile([P, NG], f32, tag="ffr" + sfx)
                nc.vector.tensor_tensor(out=fr[:], in0=lr[:], in1=adj[:],
                                        op=ALU.add)
                fl = gpool.tile([P, NG], f32, tag="ffo" + sfx)
                nc.vector.tensor_tensor(out=fl[:], in0=f0[:], in1=adj[:],
                                        op=ALU.subtract)
                return fl, fr
            y0f, ly = floor_frac(pyb, "y")
            x0f, lx = floor_frac(pxb, "x")
            omly = gpool.tile([P, NG], f32)
            omlx = gpool.tile([P, NG], f32)
            nc.vector.tensor_scalar(out=omly[:], in0=ly[:], scalar1=-1.0, scalar2=1.0,
                                    op0=ALU.mult, op1=ALU.add)
            nc.vector.tensor_scalar(out=omlx[:], in0=lx[:], scalar1=-1.0, scalar2=1.0,
                                    op0=ALU.mult, op1=ALU.add)
            # clamp biased corner coords to [-PADR, 64]+FB
            ycl = gpool.tile([P, NG], f32)
            xcl = gpool.tile([P, NG], f32)
            nc.vector.tensor_scalar(out=ycl[:], in0=y0f[:], scalar1=FB - PADR,
                                    scalar2=FB + 64.0, op0=ALU.max, op1=ALU.min)
            nc.vector.tensor_scalar(out=xcl[:], in0=x0f[:], scalar1=FB - PADR,
                                    scalar2=FB + 64.0, op0=ALU.max, op1=ALU.min)
            # row idx = (ycl-FB+PADR)*WP + (xcl-FB+PADR)
            linf = gpool.tile([P, NG], f32)
            nc.vector.scalar_tensor_tensor(
                out=linf[:], in0=ycl[:], scalar=float(WP), in1=xcl[:],
                op0=ALU.mult, op1=ALU.add)
            linf2 = gpool.tile([P, NG], f32)
            nc.vector.tensor_scalar(out=linf2[:], in0=linf[:],
                                    scalar1=-(WP + 1.0) * (FB - PADR),
                                    scalar2=None, op0=ALU.add)
            lin16 = gpool.tile([P, NG], i16)
            nc.vector.tensor_copy(out=lin16[:], in_=linf2[:])

            # weight plane products in (k, j, q) free order; per tap the
            # [128, 128] block (pos%128 x (j,q)) is PE-transposed so the DRAM
            # staging DMA is contiguous (256B/partition descriptors)
            wpre = gpool.tile([P, 4 * NG], bf16)
            wv = wpre[:].rearrange("p (k j q) -> p k j q", k=K, j=4, q=Q)
            omly3 = omly[:].rearrange("p (k q) -> p k q", k=K, q=Q)
            ly3 = ly[:].rearrange("p (k q) -> p k q", k=K, q=Q)
            omlx3 = omlx[:].rearrange("p (k q) -> p k q", k=K, q=Q)
            lx3 = lx[:].rearrange("p (k q) -> p k q", k=K, q=Q)
            nc.vector.tensor_tensor(out=wv[:, :, 0, :], in0=omly3, in1=omlx3,
                                    op=ALU.mult)  # wA = (1-ly)(1-lx)
            nc.vector.tensor_tensor(out=wv[:, :, 1, :], in0=omly3, in1=lx3,
                                    op=ALU.mult)  # wB = (1-ly)lx
            nc.vector.tensor_tensor(out=wv[:, :, 2, :], in0=ly3, in1=omlx3,
                                    op=ALU.mult)  # wC = ly(1-lx)
            nc.vector.tensor_tensor(out=wv[:, :, 3, :], in0=ly3, in1=lx3,
                                    op=ALU.mult)  # wD = ly lx
            # transpose each tap's [pos%128, (j,q)] block on PE (psum is free
            # in the gen phase), round-trip through SBUF, stage contiguously:
            # wrow[k, (j*Q + q)*128 + p]
            for k in range(K):
                pst = pspool.tile([P, NPOS], f32, tag="ps")
                nc.tensor.matmul(
                    pst[:, 0:P], wpre[:, k * P : (k + 1) * P], eye[:],
                    start=True, stop=True, skip_group_check=True)
                wtr = gpool.tile([P, P], bf16, tag="wtr")
                nc.vector.tensor_copy(out=wtr[:], in_=pst[:, 0:P])
                nc.sync.dma_start(
                    out=wrow[k : k + 1, :].rearrange("k (c p) -> k c p", c=P)[0],
                    in_=wtr[:])

            # index tensors: wrapped-16 layout for dma_gather, one tile per
            # tap so tap k's gathers only wait on tap k's build
            # idxk[k][16g + r, 8q + u] = lin16[16u + r, k*Q + q]
            idxk = []
            for k in range(K):
                t = gpool.tile([P, 8 * Q], i16, tag=f"idx{k}")
                e1 = nc.scalar if k % 2 == 0 else nc.sync
                e2 = nc.sync if k % 2 == 0 else nc.scalar
                for u in range(8):
                    e1.dma_start(
                        out=t[0:16, :].rearrange(
                            "p (q u) -> p q u", q=Q, u=8)[:, :, u],
                        in_=lin16[16 * u : 16 * u + 16,
                                  k * Q : (k + 1) * Q])
                for lo, n in [(16, 16), (32, 32), (64, 64)]:
                    e2.dma_start(out=t[lo : lo + n, :], in_=t[0:lo][0:n, :])
                idxk.append(t)

            # ------------- stage 2+3: tap-major gather/mul/matmul ----------
            # all 4 corner planes and 9 taps accumulate into the same psum
            # columns. PSUM `start` resets a whole 512-col bank, so each
            # block gets a bank-exclusive 1024-aligned stripe: blocks 0-3 in
            # generation A (8 banks), block 4 as a second 512-col generation.
            psum = pspool.tile([P, NPOS], f32, tag="ps")
            for k in range(K):
                # broadcast tap k's weight planes for blocks 0-3 (stride-0)
                wbck = wpool.tile([P, 4 * NPOS], bf16, tag="wb")
                eng = nc.sync if k % 2 == 0 else nc.scalar
                eng.dma_start(out=wbck[:],
                              in_=wrow[k : k + 1, :].to_broadcast((P, 4 * NPOS)))
                lhsT = wmat[:, k * COUT : (k + 1) * COUT]
                for blk in range(4):
                    bq, b0 = BLKQ[blk], BLK0[blk]
                    BS = bq * P  # positions in this block
                    g4 = gapool.tile([P, 4 * 896], bf16, tag="g")
                    nc.gpsimd.dma_gather(
                        g4[:, 0 : 2 * BS].rearrange("p (j n) -> p j n", j=2),
                        xr_d.ap(), idxk[k][:, 8 * b0 : 8 * (b0 + bq)],
                        BS, BS, 2 * P, transpose=True)
                    nc.gpsimd.dma_gather(
                        g4[:, 2 * BS : 4 * BS].rearrange("p (j n) -> p j n", j=2),
                        xr_d.ap()[WP:NRALLOC], idxk[k][:, 8 * b0 : 8 * (b0 + bq)],
                        BS, BS, 2 * P, transpose=True)
                    m = mpool.tile([P, 4 * 896], bf16, tag="m")
                    nc.vector.tensor_tensor(
                        out=m[:, 0 : 4 * BS].rearrange(
                            "p (j n) -> p j n", j=4),
                        in0=g4[:, 0 : 4 * BS].rearrange(
                            "p (j n) -> p j n", j=4),
                        in1=wbck[:].rearrange("p (j c) -> p j c", j=4)[
                            :, :, P * b0 : P * (b0 + bq)],
                        op=ALU.mult)
                    for j in range(4):
                        for s0, n in [(0, 512), (512, BS - 512)]:
                            nc.tensor.matmul(
                                psum[:, 1024 * blk + s0 : 1024 * blk + s0 + n],
                                lhsT, m[:, j * BS + s0 : j * BS + s0 + n],
                                start=(k == 0 and j == 0),
                                stop=(k == K - 1 and j == 3),
                                skip_group_check=True)
            # tails for blocks 0-3: bias add -> fp32 out
            for blk in range(4):
                bq, b0 = BLKQ[blk], BLK0[blk]
                BS = bq * P
                o = opool.tile([P, 896], f32, tag="o")
                nc.vector.tensor_scalar(
                    out=o[:, 0:BS], in0=psum[:, 1024 * blk : 1024 * blk + BS],
                    scalar1=bias[:, 0:1], scalar2=None, op0=ALU.add)
                nc.sync.dma_start(
                    out=out_d.ap()[:, P * b0 : P * (b0 + bq)], in_=o[:, 0:BS])
            # generation B: block 4 (512 positions, 1 bank)
            bq, b0 = BLKQ[4], BLK0[4]
            BS = bq * P
            psum2f = pspool.tile([P, NPOS], f32, tag="ps")
            psum2 = psum2f[:, 0:512]
            for k in range(K):
                wbc4 = wpool.tile([P, 4 * 512], bf16, tag="wb4")
                for j in range(4):
                    eng = nc.sync if (k + j) % 2 == 0 else nc.scalar
                    eng.dma_start(
                        out=wbc4[:, j * BS : (j + 1) * BS],
                        in_=wrow[k : k + 1,
                                 (j * Q + b0) * P : (j * Q + b0 + bq) * P]
                        .to_broadcast((P, BS)))
                g4 = gapool.tile([P, 4 * 896], bf16, tag="g")
                nc.gpsimd.dma_gather(
                    g4[:, 0 : 2 * BS].rearrange("p (j n) -> p j n", j=2),
                    xr_d.ap(), idxk[k][:, 8 * b0 : 8 * (b0 + bq)],
                    BS, BS, 2 * P, transpose=True)
                nc.gpsimd.dma_gather(
                    g4[:, 2 * BS : 4 * BS].rearrange("p (j n) -> p j n", j=2),
                    xr_d.ap()[WP:NRALLOC], idxk[k][:, 8 * b0 : 8 * (b0 + bq)],
                    BS, BS, 2 * P, transpose=True)
                m = mpool.tile([P, 4 * 896], bf16, tag="m")
                nc.vector.tensor_tensor(
                    out=m[:, 0 : 4 * BS], in0=g4[:, 0 : 4 * BS],
                    in1=wbc4[:], op=ALU.mult)
                lhsT = wmat[:, k * COUT : (k + 1) * COUT]
                for j in range(4):
                    nc.tensor.matmul(
                        psum2[:], lhsT, m[:, j * BS : (j + 1) * BS],
                        start=(k == 0 and j == 0),
                        stop=(k == K - 1 and j == 3),
                        skip_group_check=True)
            o = opool.tile([P, 896], f32, tag="o")
            nc.vector.tensor_scalar(
                out=o[:, 0:BS], in0=psum2[:],
                scalar1=bias[:, 0:1], scalar2=None, op0=ALU.add)
            nc.sync.dma_start(
                out=out_d.ap()[:, P * b0 : P * (b0 + bq)], in_=o[:, 0:BS])

    nc.compile()
    return nc


_NC_CACHE = None


def _host_inputs(x, offset, weight, bias):
    """Per-core input maps (core b <- batch b) + replicated constants."""
    wq = np.ascontiguousarray(weight, np.float32)  # [COUT, CIN, KH, KW]
    # wmat[c, k*COUT + o] = weight[o, c, ky, kx]
    wmat = wq.reshape(COUT, CIN, K).transpose(1, 2, 0).reshape(CIN, K * COUT)
    wmat = np.ascontiguousarray(wmat).astype(ml_dtypes.bfloat16)
    bias_h = np.ascontiguousarray(bias, np.float32).reshape(P, 1)
    # hob[Pp, k*Q+q] = ho(p) - 1 + ky,  wob = wo(p) - 1 + kx,  p = q*128 + Pp
    p_of = (np.arange(Q)[:, None] * P + np.arange(P)[None, :])  # [Q, P]
    ho = (p_of // WO).astype(np.float32)
    wo = (p_of % WO).astype(np.float32)
    hob = np.empty((P, K * Q), np.float32)
    wob = np.empty((P, K * Q), np.float32)
    for k in range(K):
        hob[:, k * Q : (k + 1) * Q] = (ho + (k // 3 - 1)).T
        wob[:, k * Q : (k + 1) * Q] = (wo + (k % 3 - 1)).T
    in_maps = []
    for b in range(B):
        # padded channel-minor image rows with 2-px zero ring; each stored
        # row r holds pixels (r, r+1) so a gather elem is one 512B block
        xrp = np.zeros((NRALLOC + 1, P), ml_dtypes.bfloat16)
        img = np.ascontiguousarray(x[b], np.float32)  # [C, H, W]
        xrp[:NROW].reshape(HP, WP, P)[PADR:PADR + H, PADR:PADR + W, :] = (
            img.transpose(1, 2, 0).astype(ml_dtypes.bfloat16))
        xr = np.concatenate([xrp[:NRALLOC], xrp[1:NRALLOC + 1]], axis=1)
        xr = np.ascontiguousarray(xr)
        # permuted offsets: offyx[Pp, k*Q+q (+KQ for x)] = off[2k(+1), q*128+Pp]
        off = np.ascontiguousarray(offset[b], np.float32).reshape(2 * K, Q, P)
        offyx = np.empty((P, 2 * K * Q), np.float32)
        offyx[:, 0 : K * Q] = off[0::2].transpose(2, 0, 1).reshape(P, K * Q)
        offyx[:, K * Q : 2 * K * Q] = off[1::2].transpose(2, 0, 1).reshape(P, K * Q)
        in_maps.append({
            "xr": xr,
            "offs": offyx,
            "eye": np.eye(P, dtype=ml_dtypes.bfloat16),
            "wmat": wmat,
            "bias": bias_h,
            "hob": hob,
            "wob": wob,
        })
    return in_maps


def kernel(x, offset, weight, bias):
    global _NC_CACHE
    from concourse.bass_utils import run_bass_kernel_spmd

    if _NC_CACHE is None:
        _NC_CACHE = _build_kernel()
    nc = _NC_CACHE
    in_maps = _host_inputs(x, offset, weight, bias)
    res = None
    for attempt in range(3):
        try:
            res = run_bass_kernel_spmd(nc, in_maps, list(range(B)))
            break
        except Exception:
            if attempt == 2:
                raise
    out = np.stack([res.results[b]["out"].reshape(COUT, HO, WO) for b in range(B)])
    return out.astype(np.float32)


if __name__ == "__main__":
    import sys
    d = np.load("/tmp/inputs.npz")
    if len(sys.argv) > 1 and sys.argv[1] == "sim":
        from concourse.bass_interp import CoreSim
        nc = _build_kernel()
        in_maps = _host_inputs(d["x"], d["offset"], d["weight"], d["bias"])
        sim = CoreSim(nc)
        for kk, vv in in_maps[0].items():
            sim.tensor(kk)[:] = vv
        sim.simulate()
        out = np.asarray(sim.tensor("out")).reshape(1, COUT, HO, WO)
        exp = np.load("/tmp/expected.npy")[0:1]
    else:
        out = kernel(d["x"], d["offset"], d["weight"], d["bias"])
        exp = np.load("/tmp/expected.npy")
    err = np.abs(out - exp)
    print("rel l2:", np.linalg.norm(out - exp) / np.linalg.norm(exp))
    print("absmax rel:", err.max() / np.abs(exp).max())
